# revision 1
# baseline (speedup 1.0000x reference)
"""BatchedGraphSAGEDynamicRangeMean kernel for 8 Trainium2 NeuronCores.

Sharding: data-parallel over batch b — core c computes graph c entirely
(N=4096 nodes, D=256), BN statistics are all-reduced across the 8 cores.

Per-core algorithm (all in SBUF, one pass over 32 row-blocks of 128 nodes):
  - row L2 norms -> inv norms; Xn = x * inv (normalized rows)
  - Xn^T built via PE transposes (matmul needs d on partitions)
  - per block z: banded cosine sims S = XnT[:,center]^T @ XnT[:,cand]
    (128x384, true-fp32 4-pass matmul: the 3rd/4th-neighbor margins require
    full fp32 accuracy; cand = global rows [128(z-1), 128(z+2)) )
  - window-validity additive mask, hardware max8 -> v3 = 3rd largest
  - neighbor mask C = (S >= v3) - selfdiag  (top-1 is always self: self-sim
    margin over 2nd is >=0.58 for this distribution, so C selects exactly
    the 2nd and 3rd nearest neighbors)
  - x_neib*2 = C @ x_cand  (mask matmul, float32r); the /2 is folded into Wn
  - h1 = (Xn @ WxT)*norm + bx  (un-normalization folded into the PSUM->SBUF
    copy; bias via a K=1 rank-1 matmul with inv on the lhsT)
  - h2 = x_neibT^T @ (0.5*Wn)^T + bn
  - row l2norm + relu fused into the PSUM->SBUF copies (per-partition scalars)
  - BN partial sums via ones-vector matmuls accumulated in PSUM
  - AllReduce(2x512) -> scale/bias rows -> broadcast via K=1 matmul -> apply
"""

import threading
import numpy as np

B, N, D, DOUT = 8, 4096, 256, 256
P = 128
NB = N // P            # 32 blocks
CAND = 3 * P           # 384 candidate columns per block
NCORES = 8
CH = 2 * DOUT          # 512 output channels
NEG = -1.0e30
EPS_BN = 1e-5
EPS_NORM = 1e-12

_cache = {}
_lock = threading.Lock()


def _build(single=False, phases=3):
    import concourse.bass as bass
    from concourse import bacc
    import concourse.mybir as mybir
    import concourse.tile as tile
    from concourse.masks import make_identity

    f32 = mybir.dt.float32
    f32r = mybir.dt.float32r
    AF = mybir.ActivationFunctionType
    OP = mybir.AluOpType

    nc = bacc.Bacc("TRN2", target_bir_lowering=False)
    x_in = nc.declare_dram_parameter("xb", [N, D], f32, isOutput=False)
    wxT_in = nc.declare_dram_parameter("wxT", [D, DOUT], f32, isOutput=False)
    wnTh_in = nc.declare_dram_parameter("wnTh", [D, DOUT], f32, isOutput=False)
    bx_in = nc.declare_dram_parameter("bx", [1, DOUT], f32, isOutput=False)
    bn_in = nc.declare_dram_parameter("bn", [1, DOUT], f32, isOutput=False)
    gamma_in = nc.declare_dram_parameter("gamma", [1, CH], f32, isOutput=False)
    beta_in = nc.declare_dram_parameter("beta", [1, CH], f32, isOutput=False)
    wm_in = nc.declare_dram_parameter("winmask", [P, CAND], f32, isOutput=False)
    out_ext = nc.declare_dram_parameter("out", [N, CH], f32, isOutput=True)

    with tile.TileContext(nc) as tc:
        with (
            tc.tile_pool(name="persist", bufs=1) as pp,
            tc.tile_pool(name="work", bufs=2) as wp,
            tc.tile_pool(name="ps", bufs=1, space="PSUM") as ps,
            tc.tile_pool(name="ps2", bufs=2, space="PSUM") as ps2,
            tc.tile_pool(name="ps_st", bufs=1, space="PSUM") as ps_st,
            tc.tile_pool(name="dram", bufs=1, space="DRAM") as dp,
        ):
            # ---------------- persistent tensors ----------------
            xsb = pp.tile([P, NB + 2, D], f32r)       # x rows, slot z+1 = block z
            xnT = pp.tile([P, 2, N + 2 * P], f32)     # Xn^T, col = global_row+128
            hsb = pp.tile([P, NB, CH], f32r)           # h (pre-BN)
            identity = pp.tile([P, P], f32)
            masks = pp.tile([P, CAND], f32)           # interior window mask
            diagS = pp.tile([P, CAND], f32)
            wx = pp.tile([P, 2, DOUT], f32)
            wn = pp.tile([P, 2, DOUT], f32r)
            bx_row = pp.tile([1, DOUT], f32r)
            bn_row = pp.tile([1, DOUT], f32r)
            gamma_row = pp.tile([1, CH], f32)
            beta_row = pp.tile([1, CH], f32)
            ones_row = pp.tile([1, P], f32)
            ones_row_r = pp.tile([1, P], f32r)
            ones_col = pp.tile([P, 1], f32r)
            norms = pp.tile([P, NB], f32)
            inv = pp.tile([P, NB], f32)
            invT = pp.tile([NB, P], f32)
            invT2 = pp.tile([1, NB, P], f32r)
            ssq = pp.tile([P, NB], f32)
            sbc = pp.tile([P, CH], f32)
            bbc = pp.tile([P, CH], f32)
            rowa = pp.tile([1, CH], f32)              # scratch rows, reused
            rowb = pp.tile([1, CH], f32)
            rowc = pp.tile([1, CH], f32)
            rowd = pp.tile([1, CH], f32)
            eps_t = pp.tile([1, 1], f32)

            make_identity(nc, identity)
            nc.gpsimd.memset(ones_row, 1.0)
            nc.vector.tensor_copy(ones_row_r, ones_row)
            ones_colf = pp.tile([P, 1], f32)
            nc.gpsimd.memset(ones_colf, 1.0)
            nc.vector.tensor_copy(ones_col, ones_colf)
            nc.gpsimd.memset(eps_t, EPS_BN)
            zscr = wp.tile([P, D], f32, tag="nb_sb")
            nc.gpsimd.memset(zscr, 0.0)
            nc.vector.tensor_copy(xsb[:, 0, :], zscr)
            nc.vector.tensor_copy(xsb[:, NB + 1, :], zscr)
            for c in range(2):
                nc.vector.tensor_copy(xnT[:, c, 0:P], zscr[:, 0:P])
                nc.vector.tensor_copy(xnT[:, c, N + P:N + 2 * P], zscr[:, 0:P])


            nc.sync.dma_start(masks, wm_in[:, :])
            nc.gpsimd.memset(diagS, 0.0)
            nc.vector.tensor_copy(diagS[:, P:2 * P], identity)

            for c in range(2):
                nc.sync.dma_start(wx[:, c, :], wxT_in[P * c:P * (c + 1), :])
                nc.sync.dma_start(wn[:, c, :],
                                  wnTh_in[P * c:P * (c + 1), :].bitcast(f32r))
            nc.sync.dma_start(bx_row, bx_in[:, :].bitcast(f32r))
            nc.sync.dma_start(bn_row, bn_in[:, :].bitcast(f32r))
            nc.sync.dma_start(gamma_row, gamma_in[:, :])
            nc.sync.dma_start(beta_row, beta_in[:, :])

            # ---------------- setup: load x, norms, Xn^T ----------------
            for z in range(NB):
                nc.sync.dma_start(xsb[:, z + 1, :],
                                  x_in[P * z:P * (z + 1), :].bitcast(f32r))
                xtmp = wp.tile([P, D], f32,
                               tag=("mt_sb" if z % 2 == 0 else "sim_sb"))
                nc.sync.dma_start(xtmp, x_in[P * z:P * (z + 1), :])
                xsq = wp.tile([P, D], f32,
                              tag=("hcopy" if z % 2 == 0 else "hsq"))
                nc.scalar.activation(out=xsq, in_=xtmp,
                                     func=AF.Square, accum_out=ssq[:, z:z + 1])
                nc.scalar.activation(out=norms[:, z:z + 1], in_=ssq[:, z:z + 1],
                                     func=AF.Sqrt)
                nc.vector.reciprocal(out=inv[:, z:z + 1], in_=norms[:, z:z + 1])
                xn_blk = wp.tile([P, D], f32,
                                 tag=("nb_sb" if z % 2 == 0 else "xt_sb"))
                nc.vector.tensor_scalar(out=xn_blk, in0=xtmp,
                                        scalar1=inv[:, z:z + 1], scalar2=None,
                                        op0=OP.mult)
                if z % 4 == 0:
                    tr_ps = ps2.tile([P, 2, P], f32, tag="sim")
                elif z % 4 == 1:
                    tr_ps = ps.tile([P, 2, P], f32, tag="mt")
                elif z % 4 == 2:
                    tr_ps = ps.tile([P, 2, P], f32, tag="nb")
                else:
                    tr_ps = ps.tile([P, 2, P], f32, tag="g1")
                for c in range(2):
                    nc.tensor.transpose(tr_ps[:, c, :], xn_blk[:, P * c:P * (c + 1)],
                                        identity)
                ccol = P * (z + 1)
                nc.vector.tensor_copy(xnT[:, 0, ccol:ccol + P], tr_ps[:, 0, :])
                nc.scalar.activation(out=xnT[:, 1, ccol:ccol + P], in_=tr_ps[:, 1, :],
                                     func=AF.Copy)
            trv_ps = ps.tile([NB, P], f32, tag="mt")
            nc.tensor.transpose(trv_ps, inv[:, 0:NB], identity)
            nc.vector.tensor_copy(invT, trv_ps)
            # bounce invT through DRAM so each row is addressable at partition 0
            invT_d = dp.tile([NB, P], f32)
            nc.sync.dma_start(invT_d, invT)
            nc.sync.dma_start(
                invT2,
                invT_d[:, :].rearrange("a b -> (a b)")[None, :].bitcast(f32r))

            # ---------------- main loop ----------------
            st_h = ps_st.tile([1, CH], f32, tag="sth")
            st_h2 = ps_st.tile([1, CH], f32, tag="sth2")
            for z in range(NB if phases >= 1 else 0):
                cstart = P * (z + 1)

                # banded cosine sims, true fp32 (4-pass) for top-3 accuracy.
                # Left 128 cols = transpose of previous block's right 128 cols
                # (bitwise-identical products), so compute only 256 fresh cols.
                sim_ps = ps2.tile([P, CAND], f32, tag="sim")
                if z == 0:
                    for c in range(2):
                        nc.tensor.matmul(sim_ps, xnT[:, c, cstart:cstart + P],
                                         xnT[:, c, P * z:P * z + CAND],
                                         start=(c == 0), stop=(c == 1))
                else:
                    nc.tensor.transpose(sim_ps[:, 0:P], prev_rs, identity)
                    for c in range(2):
                        nc.tensor.matmul(sim_ps[:, P:CAND],
                                         xnT[:, c, cstart:cstart + P],
                                         xnT[:, c, P * z + P:P * z + CAND],
                                         start=(c == 0), stop=(c == 1))
                if z < NB - 1:
                    prev_rs = wp.tile([P, P], f32, tag="right")
                    nc.scalar.activation(out=prev_rs, in_=sim_ps[:, 2 * P:CAND],
                                         func=AF.Copy)
                sim_sb = wp.tile([P, CAND], f32, tag="sim_sb")
                nc.vector.tensor_add(sim_sb[:, 0:P], sim_ps[:, 0:P],
                                     masks[:, 0:P])
                nc.vector.tensor_add(sim_sb[:, P:CAND], sim_ps[:, P:CAND],
                                     masks[:, P:CAND])
                if z == 0:
                    nc.vector.tensor_scalar_add(sim_sb[:, 0:P], sim_sb[:, 0:P], NEG)
                elif z == NB - 1:
                    nc.vector.tensor_scalar_add(sim_sb[:, 2 * P:CAND],
                                                sim_sb[:, 2 * P:CAND], NEG)

                top8 = wp.tile([P, 8], f32, tag="top8")
                nc.vector.max(out=top8, in_=sim_sb)
                maskc = sim_sb
                for k in range(3):
                    sl = slice(P * k, P * (k + 1))
                    nc.vector.scalar_tensor_tensor(out=maskc[:, sl],
                                                   in0=sim_sb[:, sl],
                                                   scalar=top8[:, 2:3],
                                                   in1=diagS[:, sl],
                                                   op0=OP.is_ge, op1=OP.subtract)

                mt_ps = ps.tile([P, CAND], f32, tag="mt")
                for k in range(3):
                    nc.tensor.transpose(mt_ps[:, P * k:P * (k + 1)],
                                        maskc[:, P * k:P * (k + 1)], identity)
                mt_sb = wp.tile([P, CAND], f32r, tag="mt_sb")
                nc.scalar.activation(out=mt_sb, in_=mt_ps, func=AF.Copy)

                # x_neib*2 = C @ x_cand  (natural layout), then transpose
                nb_ps = ps.tile([P, D], f32, tag="nb")
                for k in range(3):
                    nc.tensor.matmul(nb_ps, mt_sb[:, P * k:P * (k + 1)],
                                     xsb[:, z + k, :],
                                     start=(k == 0), stop=(k == 2))
                nb_sb = wp.tile([P, D], f32, tag="nb_sb")
                nc.scalar.activation(out=nb_sb, in_=nb_ps, func=AF.Copy)
                xt_ps = ps.tile([P, 2, P], f32, tag="nb")
                for c in range(2):
                    nc.tensor.transpose(xt_ps[:, c, :], nb_sb[:, P * c:P * (c + 1)],
                                        identity)
                xt_sb = wp.tile([P, 2, P], f32r, tag="xt_sb")
                nc.vector.tensor_copy(xt_sb, xt_ps)

                # g1 = Xn @ WxT (+ inv*bx rank-1) ; h2 = x_neibT^T @ WnT_half + bn
                g1_ps = ps.tile([P, DOUT], f32, tag="g1")
                for c in range(2):
                    nc.tensor.matmul(g1_ps, xnT[:, c, cstart:cstart + P],
                                     wx[:, c, :], start=(c == 0), stop=False)
                nc.tensor.matmul(g1_ps, invT2[:, z, :], bx_row,
                                 start=False, stop=True)
                h2_ps = ps.tile([P, DOUT], f32, tag="h2")
                for c in range(2):
                    nc.tensor.matmul(h2_ps, xt_sb[:, c, :], wn[:, c, :],
                                     start=(c == 0), stop=False)
                nc.tensor.matmul(h2_ps, ones_row_r, bn_row,
                                 start=False, stop=True)

                # fused l2norm + relu on the way out of PSUM
                sq_scr = wp.tile([P, DOUT], f32, tag="hsq")
                sA = wp.tile([P, 1], f32, tag="sA")
                nc.scalar.activation(out=sq_scr, in_=g1_ps, func=AF.Square,
                                     accum_out=sA)
                sq_scr2 = wp.tile([P, DOUT], f32, tag="hsq")
                sB = wp.tile([P, 1], f32, tag="sB")
                nc.scalar.activation(out=sq_scr2, in_=h2_ps, func=AF.Square,
                                     accum_out=sB)
                tot = wp.tile([P, 1], f32, tag="tot")
                nc.vector.scalar_tensor_tensor(out=tot, in0=sA,
                                               scalar=ssq[:, z:z + 1], in1=sB,
                                               op0=OP.mult, op1=OP.add)
                hno = wp.tile([P, 1], f32, tag="hno")
                nc.scalar.activation(out=hno, in_=tot, func=AF.Sqrt)
                nc.vector.tensor_scalar_max(hno, hno, EPS_NORM)
                rinv = wp.tile([P, 1], f32, tag="rinv")
                nc.vector.reciprocal(out=rinv, in_=hno)
                s1 = wp.tile([P, 1], f32, tag="s1")
                nc.vector.tensor_mul(s1, norms[:, z:z + 1], rinv)
                nc.scalar.activation(out=hsb[:, z, 0:DOUT], in_=g1_ps,
                                     func=AF.Relu, scale=s1)
                nc.scalar.activation(out=hsb[:, z, DOUT:CH], in_=h2_ps,
                                     func=AF.Relu, scale=rinv)

                # BN partial sums (accumulated in PSUM across all blocks)
                hsq = wp.tile([P, CH], f32r, tag="hsq")
                nc.scalar.activation(out=hsq, in_=hsb[:, z, :].bitcast(f32),
                                     func=AF.Square)
                nc.tensor.matmul(st_h, ones_col, hsb[:, z, :],
                                 start=(z == 0), stop=(z == NB - 1))
                nc.tensor.matmul(st_h2, ones_col, hsq,
                                 start=(z == 0), stop=(z == NB - 1))

            # ---------------- BN stats all-reduce ----------------
            if phases >= 1:
                nc.vector.tensor_copy(rowa, st_h)
                nc.vector.tensor_copy(rowb, st_h2)
            else:
                nc.vector.memset(rowa, 0.0)
                nc.vector.memset(rowb, 1.0)
            st_in_d = dp.tile([2, CH], f32)
            st_out_d = dp.tile([2, CH], f32)
            nc.sync.dma_start(st_in_d[0:1, :], rowa)
            nc.sync.dma_start(st_in_d[1:2, :], rowb)
            if single:
                nc.sync.dma_start(st_out_d, st_in_d[:, :])
            else:
                nc.gpsimd.collective_compute(
                    "AllReduce", mybir.AluOpType.add,
                    replica_groups=[list(range(NCORES))],
                    ins=[st_in_d[:].opt()],
                    outs=[st_out_d[:].opt()],
                )
            nc.sync.dma_start(rowa, st_out_d[0:1, :])   # rowa = sum h
            nc.sync.dma_start(rowb, st_out_d[1:2, :])   # rowb = sum h^2
            sc = 1.0 / float(B * N)
            nc.vector.tensor_scalar_mul(rowa, rowa, sc)       # mu
            nc.vector.tensor_scalar_mul(rowb, rowb, sc)       # E[h^2]
            nc.vector.tensor_mul(rowc, rowa, rowa)            # mu^2
            nc.vector.tensor_sub(rowb, rowb, rowc)            # var
            nc.scalar.activation(out=rowb, in_=rowb, func=AF.Sqrt, bias=eps_t)
            nc.vector.reciprocal(out=rowb, in_=rowb)          # rstd
            nc.vector.tensor_mul(rowb, rowb, gamma_row)       # s = gamma*rstd
            nc.vector.tensor_mul(rowc, rowa, rowb)            # mu*s
            nc.vector.tensor_sub(rowd, beta_row, rowc)        # b = beta - mu*s

            # broadcast scale/bias rows to 128 partitions via K=1 matmul
            bc_ps = ps2.tile([P, CH], f32, tag="sim")
            nc.tensor.matmul(bc_ps, ones_row, rowb, start=True, stop=True)
            nc.vector.tensor_copy(sbc, bc_ps)
            bc_ps2 = ps.tile([P, CH], f32, tag="mt")
            nc.tensor.matmul(bc_ps2, ones_row, rowd, start=True, stop=True)
            nc.vector.tensor_copy(bbc, bc_ps2)

            # ---------------- BN apply + writeback ----------------
            for z in range(NB if phases >= 2 else 0):
                out_t = wp.tile([P, CH], f32, tag=("hcopy" if z % 2 == 0 else "hsq"))
                if z % 4 != 3:
                    nc.vector.tensor_mul(out_t, hsb[:, z, :].bitcast(f32), sbc)
                    nc.vector.tensor_add(out_t, out_t, bbc)
                else:
                    nc.gpsimd.tensor_mul(out_t, hsb[:, z, :].bitcast(f32), sbc)
                    nc.vector.tensor_add(out_t, out_t, bbc)
                nc.sync.dma_start(out_ext[P * z:P * (z + 1), :], out_t)

    return _finish(nc)


def _finish(nc):
    nc.finalize()
    return nc


def _get_nc():
    with _lock:
        if "nc" not in _cache:
            _cache["nc"] = _build()
        return _cache["nc"]


def _run(inputs, trace=False, trace_kwargs=None):
    from concourse.bass_utils import run_bass_kernel_spmd

    x = np.ascontiguousarray(np.asarray(inputs["x"], dtype=np.float32))
    Wx_w = np.asarray(inputs["Wx_w"], dtype=np.float32)
    Wx_b = np.asarray(inputs["Wx_b"], dtype=np.float32)
    Wn_w = np.asarray(inputs["Wn_w"], dtype=np.float32)
    Wn_b = np.asarray(inputs["Wn_b"], dtype=np.float32)
    gamma = np.asarray(inputs["gamma"], dtype=np.float32)
    beta = np.asarray(inputs["beta"], dtype=np.float32)
    assert x.shape == (B, N, D), x.shape
    assert int(inputs["p"]) == 16 and int(inputs["t"]) == 8

    wxT = np.ascontiguousarray(Wx_w.T)
    wnTh = np.ascontiguousarray((0.5 * Wn_w).T)
    wm = np.full((P, CAND), NEG, dtype=np.float32)
    for j in range(8):
        wm[16 * j:16 * j + 16, 16 * j:16 * j + 272] = 0.0
    shared = {
        "wxT": wxT, "wnTh": wnTh, "winmask": wm,
        "bx": Wx_b.reshape(1, DOUT), "bn": Wn_b.reshape(1, DOUT),
        "gamma": gamma.reshape(1, CH), "beta": beta.reshape(1, CH),
    }
    in_maps = [{"xb": np.ascontiguousarray(x[c]), **shared} for c in range(NCORES)]

    nc = _get_nc()
    kw = {}
    if trace:
        kw = dict(trace=True, trace_kwargs=trace_kwargs or {})
    res = run_bass_kernel_spmd(nc, in_maps, core_ids=list(range(NCORES)), **kw)
    out = np.stack([res.results[c]["out"] for c in range(NCORES)], axis=0)
    return out.astype(np.float32), res


def kernel(**inputs):
    out, _ = _run(inputs)
    return out

